# revision 1
# baseline (speedup 1.0000x reference)
"""MoE clustered attention kernel for Trainium2 (8 NeuronCores).

Problem: B=2, LQ=LK=2048, D=1024, H=16 heads (DH=64), M=8 clusters.
Each query/key token is routed (argmax of X @ Wr) to one of 8 clusters;
attention is only computed within a cluster (block-sparse attention).

Strategy
--------
Host side:
  * compute router assignments with numpy fp32 (verified to match the
    jax reference on every argmax decision; min top-2 logit gap for
    these inputs is 1.5e-4, far above fp32/bf16 rounding noise),
  * gather tokens by cluster so in-cluster attention becomes block
    attention on contiguous ranges; pad each cluster to a common
    cross-batch geometry (queries to >=256 and even, keys to multiples
    of 128) so one SPMD program serves both batches,
  * pre-transpose X to [D, L] so on-device projections contract over
    the partition dimension directly,
  * append 9 "mask rows" to the per-head qT/kT tensors: the scores
    matmul contracts over 64+9=73 rows and the extra rows add exactly
    0 to same-cluster pairs and exactly -16384 to cross-cluster or
    padded pairs (exp(x-16384) == 0), which makes every (k-slice,
    q-slice) block correct regardless of cluster boundaries and
    handles padding for free (all mask constants are powers of two,
    exact in bf16, and cancel exactly inside the matmul).

Device side (per core; core = batch * 4 + head_group, 4 heads each):
  * per-head qT/kT projections in transposed layout [73, L],
  * v in natural layout [tokens, 4 heads x (64 + ones col)]; the ones
    column makes the ctx matmul emit the softmax denominator as row 64,
  * per (head, cluster): one K=73 scores matmul per 128-key chunk,
    exp on ScalarE (no max-subtraction needed: scores are O(30); the
    masked entries underflow to exactly 0), ctx matmul accumulation,
    denominator rows staged into a packed [32, 512] tensor,
  * one batched reciprocal over all 32 denominator rows, GpSimd
    partition-broadcast per (head, cluster), in-place normalize on
    VectorE, then the output projection (partial over 4 heads).
Host sums the 4 head-group partials per batch and un-permutes rows.

Matmul dtype is bf16 by default (fp32 PSUM accumulation; measured end
to end relative error ~4.6e-3). Set BASS_MM_DTYPE=f32r for the fp32r
variant (~3e-4, ~1.5x slower).
"""

import os

import numpy as np
import ml_dtypes

import concourse.bacc as bacc
import concourse.tile as tile
import concourse.mybir as mybir
from concourse.bass_utils import run_bass_kernel_spmd

F32 = mybir.dt.float32
F32R = mybir.dt.float32r
BF16 = mybir.dt.bfloat16
EXP = mybir.ActivationFunctionType.Exp
MULT = mybir.AluOpType.mult

H = 16            # total heads
HPC = 4           # heads per core
N_CORES = 8
SQRT_BIG = 128.0  # sqrt(16384); mask contributions are exact powers of two

MMDT = F32R if os.environ.get("BASS_MM_DTYPE") == "f32r" else BF16


def _ceil_to(x, m):
    return (x + m - 1) // m * m


def _plan(aq, ak, M):
    """Common (cross-batch) padded cluster geometry."""
    B = aq.shape[0]
    nq = np.array([[int((aq[b] == c).sum()) for c in range(M)] for b in range(B)])
    nk = np.array([[int((ak[b] == c).sum()) for c in range(M)] for b in range(B)])
    # fp32r matmuls require an even moving free dim -> round up to even
    NQP = [max(256, _ceil_to(int(nq[:, c].max()), 2)) for c in range(M)]
    NKP = [_ceil_to(max(128, int(nk[:, c].max())), 128) for c in range(M)]
    qoff = np.concatenate([[0], np.cumsum(NQP)])
    koff = np.concatenate([[0], np.cumsum(NKP)])
    LQG = _ceil_to(int(qoff[-1]), 256)
    NKG = _ceil_to(int(koff[-1]), 256)
    return NQP, NKP, qoff[:-1].tolist(), koff[:-1].tolist(), LQG, NKG


def _build_program(NQP, NKP, qoffs, koffs, LQG, NKG, D):
    nc = bacc.Bacc("TRN2", target_bir_lowering=False, debug=False)
    XQT = nc.dram_tensor("XQT", [D, LQG], MMDT, kind="ExternalInput").ap()
    XKT = nc.dram_tensor("XKT", [D, NKG], MMDT, kind="ExternalInput").ap()
    XVT = nc.dram_tensor("XVT", [D, NKG], MMDT, kind="ExternalInput").ap()
    WQ = nc.dram_tensor("WQ", [D, 256], MMDT, kind="ExternalInput").ap()
    WK = nc.dram_tensor("WK", [D, 256], MMDT, kind="ExternalInput").ap()
    WV = nc.dram_tensor("WV", [D, 256], MMDT, kind="ExternalInput").ap()
    WO = nc.dram_tensor("WO", [256, D], MMDT, kind="ExternalInput").ap()
    MQ = nc.dram_tensor("MQ", [9, LQG], MMDT, kind="ExternalInput").ap()
    MK = nc.dram_tensor("MK", [9, NKG], MMDT, kind="ExternalInput").ap()
    OUT = nc.dram_tensor("OUT", [LQG, D], F32, kind="ExternalOutput").ap()

    ND = D // 128          # contraction chunks (8)
    NVC = NKG // 128       # value token chunks
    M = len(NQP)

    def ms_view(ap):
        # memset can't write fp32r dtype; write the same bits as fp32
        return ap.bitcast(F32) if MMDT == F32R else ap

    with tile.TileContext(nc) as tc:
        with (
            tc.tile_pool(name="weights", bufs=1) as wpool,
            tc.tile_pool(name="proj_out", bufs=1) as projpool,
            tc.tile_pool(name="psA", bufs=2, space="PSUM") as psA,
            tc.tile_pool(name="psB", bufs=2, space="PSUM") as psB,
            tc.tile_pool(name="psC", bufs=2, space="PSUM") as psC,
        ):
            wq = wpool.tile([128, ND * 256], MMDT, tag="wq")
            wk = wpool.tile([128, ND * 256], MMDT, tag="wk")
            wv = wpool.tile([128, ND * 256], MMDT, tag="wv")
            wo = wpool.tile([128, 2 * 1024], MMDT, tag="wo")
            nc.sync.dma_start(wq[:].rearrange("p (n m) -> p n m", n=ND),
                              WQ.rearrange("(n p) m -> p n m", p=128))
            nc.sync.dma_start(wk[:].rearrange("p (n m) -> p n m", n=ND),
                              WK.rearrange("(n p) m -> p n m", p=128))
            nc.sync.dma_start(wv[:].rearrange("p (n m) -> p n m", n=ND),
                              WV.rearrange("(n p) m -> p n m", p=128))
            nc.sync.dma_start(wo[:].rearrange("p (n m) -> p n m", n=2),
                              WO.rearrange("(n p) m -> p n m", p=128))

            # per-head [73, L]: rows 0..63 head dims, rows 64..72 mask rows
            qT = [projpool.tile([73, LQG], MMDT, tag=f"qT{h}", name=f"qT{h}")
                  for h in range(HPC)]
            kT = [projpool.tile([73, NKG], MMDT, tag=f"kT{h}", name=f"kT{h}")
                  for h in range(HPC)]
            vA = projpool.tile([128, NVC * 260], MMDT, tag="vA")
            ctxT = [projpool.tile([128, LQG], MMDT, tag=f"ctxT{p}", name=f"ctxT{p}")
                    for p in range(2)]
            # denominator rows: head h's clusters at quad-aligned rows 32h..32h+M
            dn = projpool.tile([128, 512], F32, tag="dn")
            rcp = projpool.tile([128, 512], F32, tag="rcp")

            for h in range(HPC):
                nc.sync.dma_start(qT[h][64:73, :], MQ)
                nc.sync.dma_start(kT[h][64:73, :], MK)

            # ones columns of v_aug (col 64 of each head's 65-wide block):
            # memset everything to 1.0; the projection copies below
            # overwrite the 4x64 value columns, leaving col 64 at 1.0.
            nc.vector.memset(ms_view(vA[:]), 1.0)
            # zero the tail columns of ctxT that attention never writes
            tail = int(np.sum(NQP))
            if tail < LQG:
                for p in range(2):
                    nc.vector.memset(ms_view(ctxT[p][:, tail:LQG]), 0.0)

            # ---- projections ----
            with tc.tile_pool(name="xin", bufs=3) as xpool:
                def proj_T(xdram, L, wtile, dest):
                    """dest[h][0:64, L] = (W_h.T @ X^T), streamed over L."""
                    for off in range(0, L, 512):
                        w = min(512, L - off)
                        xt = xpool.tile([128, ND, 512], MMDT, tag="xt")
                        nc.sync.dma_start(
                            xt[:, :, :w],
                            xdram.rearrange("(n p) m -> p n m", p=128)[:, :, off:off + w])
                        for pair in range(2):
                            ps = psA.tile([128, 512], F32, tag="psproj")
                            for half in range(0, w, 256):
                                w2 = min(256, w - half)
                                for d in range(ND):
                                    nc.tensor.matmul(
                                        ps[:, half:half + w2],
                                        wtile[:, d * 256 + pair * 128: d * 256 + (pair + 1) * 128],
                                        xt[:, d, half:half + w2],
                                        start=(d == 0), stop=(d == ND - 1))
                            if pair == 0:
                                nc.vector.tensor_copy(dest[0][0:64, off:off + w], ps[0:64, :w])
                                nc.scalar.copy(dest[1][0:64, off:off + w], ps[64:128, :w])
                            else:
                                nc.scalar.copy(dest[2][0:64, off:off + w], ps[0:64, :w])
                                nc.vector.tensor_copy(dest[3][0:64, off:off + w], ps[64:128, :w])

                proj_T(XQT, LQG, wq, qT)
                proj_T(XKT, NKG, wk, kT)

                # value projection: natural layout, 4 heads + ones col
                for off in range(0, NKG, 512):
                    w = min(512, NKG - off)
                    xt = xpool.tile([128, ND, 512], MMDT, tag="xt")
                    nc.sync.dma_start(
                        xt[:, :, :w],
                        XVT.rearrange("(n p) m -> p n m", p=128)[:, :, off:off + w])
                    for sub in range(w // 128):
                        tc128 = off // 128 + sub
                        ps = psA.tile([128, 256], F32, tag="psproj")
                        for d in range(ND):
                            nc.tensor.matmul(ps[:],
                                             xt[:, d, sub * 128:(sub + 1) * 128],
                                             wv[:, d * 256:(d + 1) * 256],
                                             start=(d == 0), stop=(d == ND - 1))
                        nc.vector.tensor_copy(
                            vA[:].rearrange("p (c h e) -> p c h e", c=NVC, h=HPC)[:, tc128, :, 0:64],
                            ps[:].rearrange("p (h e) -> p h e", h=HPC))

            # ---- clustered attention ----
            with tc.tile_pool(name="epool", bufs=3) as epool, \
                 tc.tile_pool(name="btpool", bufs=4) as btpool:
                for h in range(HPC):
                    pair, rb = h // 2, (h % 2) * 64
                    for c in range(M):
                        qo, nqp = qoffs[c], NQP[c]
                        nkc = NKP[c] // 128
                        # scores into 2-bank super tiles; exp reads pairs of
                        # banks in one strided ACTIVATE (halves ACT overhead)
                        sts, es, eslice = [], [], []
                        for ki in range(0, nkc, 2):
                            nk2 = min(2, nkc - ki)
                            ps_s = psB.tile([128, 1024], F32, tag="ps_s")
                            e = epool.tile([128, 1024], MMDT, tag="e")
                            for kj in range(nk2):
                                ko = koffs[c] + (ki + kj) * 128
                                nc.tensor.matmul(
                                    ps_s[:, kj * 512: kj * 512 + nqp],
                                    kT[h][0:73, ko:ko + 128],
                                    qT[h][0:73, qo:qo + nqp],
                                    start=True, stop=True)
                                es.append(e)
                                eslice.append(slice(kj * 512, kj * 512 + nqp))
                            pv = ps_s[:].rearrange("p (b n) -> p b n", b=2)[:, 0:nk2, 0:nqp]
                            ev = e[:].rearrange("p (b n) -> p b n", b=2)[:, 0:nk2, 0:nqp]
                            nc.scalar.activation(ev, pv, EXP)
                        ps_c = psC.tile([128, 512], F32, tag="ps_c")
                        for ki in range(nkc):
                            kc128 = koffs[c] // 128 + ki
                            nc.tensor.matmul(ps_c[:65, :nqp],
                                             vA[:, kc128 * 260 + h * 65: kc128 * 260 + (h + 1) * 65],
                                             es[ki][:, eslice[ki]],
                                             start=(ki == 0), stop=(ki == nkc - 1))
                        # evacuate unnormalized ctx; stage denominator row
                        # into this head's quad of `dn` (DMA writes may target
                        # any partition; compute engines may not)
                        nc.vector.tensor_copy(ctxT[pair][rb:rb + 64, qo:qo + nqp],
                                              ps_c[0:64, :nqp])
                        stg = btpool.tile([1, 512], F32, tag="stg")
                        nc.scalar.copy(stg[:, :nqp], ps_c[64:65, :nqp])
                        nc.sync.dma_start(dn[32 * h + c: 32 * h + c + 1, :nqp],
                                          stg[:, :nqp])
                    # all of head h's denominators staged: one batched recip,
                    # then normalize in place (overlaps head h+1's attention)
                    nc.vector.reciprocal(rcp[32 * h: 32 * h + M, :],
                                         dn[32 * h: 32 * h + M, :])
                    for c in range(M):
                        qo, nqp = qoffs[c], NQP[c]
                        btsrc = btpool.tile([1, 512], F32, tag="btsrc")
                        nc.sync.dma_start(btsrc[:, :nqp],
                                          rcp[32 * h + c: 32 * h + c + 1, :nqp])
                        bt = btpool.tile([128, 512], F32, tag="bt")
                        nc.gpsimd.partition_broadcast(bt[:, :nqp], btsrc[:, :nqp])
                        nc.vector.tensor_tensor(ctxT[pair][rb:rb + 64, qo:qo + nqp],
                                                ctxT[pair][rb:rb + 64, qo:qo + nqp],
                                                bt[rb:rb + 64, :nqp], MULT)

            # ---- output projection ----
            with tc.tile_pool(name="outsb", bufs=4) as opool:
                for mi in range(LQG // 128):
                    for n2 in range(2):
                        ps_o = psB.tile([128, 512], F32, tag="ps_s")
                        for half in range(2):
                            for pair in range(2):
                                nc.tensor.matmul(
                                    ps_o[:, half * 256:(half + 1) * 256],
                                    ctxT[pair][:, mi * 128:(mi + 1) * 128],
                                    wo[:, pair * 1024 + n2 * 512 + half * 256:
                                       pair * 1024 + n2 * 512 + (half + 1) * 256],
                                    start=(pair == 0), stop=(pair == 1))
                        ob = opool.tile([128, 512], F32, tag="ob")
                        if (mi + n2) % 2:
                            nc.scalar.copy(ob[:], ps_o[:])
                        else:
                            nc.vector.tensor_copy(ob[:], ps_o[:])
                        nc.sync.dma_start(
                            OUT[mi * 128:(mi + 1) * 128, n2 * 512:(n2 + 1) * 512], ob[:])

    nc.compile()
    return nc


_CACHE = {}


def run(inputs, trace=False):
    queries = np.asarray(inputs["queries"], np.float32)
    keys = np.asarray(inputs["keys"], np.float32)
    values = np.asarray(inputs["values"], np.float32)
    Wq = np.asarray(inputs["Wq"], np.float32)
    Wk = np.asarray(inputs["Wk"], np.float32)
    Wv = np.asarray(inputs["Wv"], np.float32)
    Wo = np.asarray(inputs["Wo"], np.float32)
    Wr = np.asarray(inputs["Wr"], np.float32)

    B, LQ, D = queries.shape
    M = Wr.shape[1]
    DH = D // H
    scale = np.float32(1.0 / np.sqrt(DH))
    npdt = ml_dtypes.bfloat16 if MMDT == BF16 else np.float32

    aq = np.argmax(queries @ Wr, axis=-1)   # [B, LQ]
    ak = np.argmax(keys @ Wr, axis=-1)      # [B, LK]

    NQP, NKP, qoffs, koffs, LQG, NKG = _plan(aq, ak, M)

    key = (tuple(NQP), tuple(NKP), LQG, NKG, D, str(MMDT))
    if key not in _CACHE:
        _CACHE[key] = _build_program(NQP, NKP, qoffs, koffs, LQG, NKG, D)
    nc = _CACHE[key]

    # ---- gather + pad, build per-batch inputs ----
    perm_q = []   # original token ids, per batch, in gathered order
    slot_q = []   # gathered positions of those tokens
    XQTs, XKTs, XVTs, MQs, MKs = [], [], [], [], []
    for b in range(B):
        xq = np.zeros((LQG, D), np.float32)
        xk = np.zeros((NKG, D), np.float32)
        xv = np.zeros((NKG, D), np.float32)
        mqa = np.zeros((9, LQG), np.float32)
        mka = np.zeros((9, NKG), np.float32)
        mka[8, :] = SQRT_BIG
        pq, sq = [], []
        for c in range(M):
            tq = np.nonzero(aq[b] == c)[0]
            tk = np.nonzero(ak[b] == c)[0]
            xq[qoffs[c]:qoffs[c] + len(tq)] = queries[b, tq]
            xk[koffs[c]:koffs[c] + len(tk)] = keys[b, tk]
            xv[koffs[c]:koffs[c] + len(tk)] = values[b, tk]
            mqa[c, qoffs[c]:qoffs[c] + len(tq)] = SQRT_BIG
            mqa[8, qoffs[c]:qoffs[c] + len(tq)] = -SQRT_BIG
            mka[c, koffs[c]:koffs[c] + len(tk)] = SQRT_BIG
            pq.append(tq)
            sq.append(np.arange(qoffs[c], qoffs[c] + len(tq)))
        perm_q.append(np.concatenate(pq))
        slot_q.append(np.concatenate(sq))
        XQTs.append(np.ascontiguousarray(xq.T).astype(npdt))
        XKTs.append(np.ascontiguousarray(xk.T).astype(npdt))
        XVTs.append(np.ascontiguousarray(xv.T).astype(npdt))
        MQs.append(mqa.astype(npdt))
        MKs.append(mka.astype(npdt))

    in_maps = []
    for core in range(N_CORES):
        b, hg = core // HPC, core % HPC
        cols = slice(hg * HPC * DH, (hg + 1) * HPC * DH)
        in_maps.append({
            "XQT": XQTs[b], "XKT": XKTs[b], "XVT": XVTs[b],
            "WQ": np.ascontiguousarray(Wq[:, cols] * scale).astype(npdt),
            "WK": np.ascontiguousarray(Wk[:, cols]).astype(npdt),
            "WV": np.ascontiguousarray(Wv[:, cols]).astype(npdt),
            "WO": np.ascontiguousarray(Wo[cols, :]).astype(npdt),
            "MQ": MQs[b], "MK": MKs[b],
        })

    res = run_bass_kernel_spmd(nc, in_maps, list(range(N_CORES)), trace=trace)

    out = np.zeros((B, LQ, D), np.float32)
    for b in range(B):
        acc = res.results[b * HPC]["OUT"].copy()
        for hg in range(1, HPC):
            acc += res.results[b * HPC + hg]["OUT"]
        out[b, perm_q[b]] = acc[slot_q[b]]
    return out, res


def kernel(**inputs):
    out, _ = run(inputs)
    return out

